# revision 2
# baseline (speedup 1.0000x reference)
"""GraphSAGE 2-layer mean-aggregation kernel for 8 Trainium2 NeuronCores.

Problem (full shapes):
    features [2_000_000, 128] f32, samples0 [1024], samples1 [1024, 25],
    samples2 [1024, 25, 10] -> out [1024, 256] f32.

Strategy (fp8 feature stream; PE projects, DVE fuses relu+accumulate):
  * Data-parallel over the batch: core c handles batches [128c, 128c+128).
  * Per the sharding_hint's all-to-all gather, each core is staged exactly
    the rows its samples reference — TRANSPOSED (feature-major) in fp8e4
    (hop-2 4.1 MB/core + hop-1 0.41 MB/core); hop-0 and the layer-1
    weights stay fp16.  ~4.8 MB/core vs 9.3 MB for the fp16 baseline,
    which halves the DMA stream this memory-bound kernel rides on.
  * The s2-mean and the s1-mean of h1 fuse into the PE as fp8 DoubleRow
    matmuls (two feature planes per pass with the scaled aggregation
    weight duplicated into both k-tiles): a 10-plane s2-mean is 5 passes.
    The back-to-back matmul stream keeps the PE at its ramped clock.
  * Each 2-slice h2 chunk produces ONE 512-col PSUM tile [half, slice,
    batch]; a single DVE scalar_tensor_tensor per chunk then does
    acc += max(psum, 0) — fused relu + s1-accumulation into a parity-
    split fp16 accumulator.  One cross-engine hop per chunk; the DVE
    chain trails the stream by its own throughput only, so the multi-
    microsecond first-dependency semaphore latency of this stack is
    paid once, not per chunk.
  * w8 + h1 ride at the FRONT of the sync queue (strict FIFO beats the
    bursty cross-queue DMA-engine arbitration on HW); h0w on the scalar
    queue.  The 1/SCL of the fp8 weight scaling folds into the layer-1
    wn1 weights on the host.  Tail after the last (single-slice) chunk:
    1 DVE op + 4 matmuls + 1 relu + one 64 KB output DMA.

Self-contained: hardcodes all shapes; only needs numpy + ml_dtypes + the
concourse (Bass) stack on the container's default python path.
"""

import sys

for _p in ("/opt/trn_rl_repo",):
    if _p not in sys.path:
        sys.path.append(_p)

import ml_dtypes
import numpy as np

import concourse.bass as bass
import concourse.mybir as mybir
import concourse.tile as tile
from concourse import bacc
from concourse.bass_utils import run_bass_kernel_spmd

F32 = mybir.dt.float32
F16 = mybir.dt.float16
F8 = mybir.dt.float8e4
RELU = mybir.ActivationFunctionType.Relu
DROW = mybir.MatmulPerfMode.DoubleRow
MAX = mybir.AluOpType.max
ADD = mybir.AluOpType.add

N_CORES = 8
B = 1024
BL = B // N_CORES          # 128 batches per core
S1, S2 = 25, 10
D = 128                    # feature dim = OUT0 = OUT1 = 128
SCL = 64.0                 # fp8 weight scale (folded into wn1 / relus)
CHUNK_SLS = (2,) * 12 + (1,)
CHUNK_OFF = tuple(range(0, 24, 2)) + (24,)
H2_CHUNKS = len(CHUNK_SLS)            # 13
N_H1 = BL * S1                        # 3200 cols
N_H2 = BL * S1 * S2                   # 32000 cols
W_NAMES = ("ws0", "ws1a", "ws1b", "wn1a", "wn1b")


def build_bass() -> bass.Bass:
    nc = bacc.Bacc()

    # feature-major staged tables: row = feature.
    # h2 cols per chunk are (s2, s1_local, batch); h1 cols are (s1, batch).
    h2_d = nc.dram_tensor("h2", [D, N_H2], F8, kind="ExternalInput")
    h1_d = nc.dram_tensor("h1", [D, N_H1], F8, kind="ExternalInput")
    # [h0T (128 cols) | 5 fp16 weight blocks of 128 cols]
    h0w_d = nc.dram_tensor("h0w", [D, (1 + len(W_NAMES)) * D], F16,
                           kind="ExternalInput")
    # fp8 scaled aggregation weights:
    #   [0:2] wn0*SCL/S2 duplicated (DoubleRow s2 pairs)
    #   [2]   ws0*SCL
    #   [3:5] wn0*SCL/S1 duplicated (DoubleRow s1 pairs)
    w8_d = nc.dram_tensor("w8", [D, 5, D], F8, kind="ExternalInput")
    out_d = nc.dram_tensor("out", [D, 2 * BL], F16, kind="ExternalOutput")

    with tile.TileContext(nc) as tc:
        with (
            tc.tile_pool(name="const", bufs=1) as cpool,
            tc.tile_pool(name="h2", bufs=13) as h2pool,
            tc.tile_pool(name="ps", bufs=5, space="PSUM") as pspool,
            tc.tile_pool(name="psa", bufs=1, space="PSUM") as psapool,
        ):
            def dma_h2(c):
                nsl = CHUNK_SLS[c]
                t = h2pool.tile([D, S2, nsl, BL], F8, tag="h2c")
                c0 = CHUNK_OFF[c] * S2 * BL
                nc.sync.dma_start(
                    t[:],
                    h2_d[:, c0:c0 + nsl * S2 * BL].rearrange(
                        "p (t s b) -> p t s b", t=S2, s=nsl),
                )
                return t

            # w8 + h1 at the FRONT of the sync queue: strict FIFO with the
            # h2 chunks guarantees they land first (cross-queue DMA-engine
            # arbitration on HW is bursty, multi-microsecond).  h0w rides
            # the scalar queue, interleaving with the early stream.
            w8 = cpool.tile([D, 5, D], F8, tag="w8")
            nc.sync.dma_start(w8[:], w8_d[:])
            h1t = cpool.tile([D, S1, BL], F8, tag="h1")
            nc.sync.dma_start(
                h1t[:], h1_d[:].rearrange("p (s b) -> p s b", s=S1))
            h0w = cpool.tile([D, (1 + len(W_NAMES)) * D], F16, tag="h0w")
            nc.scalar.dma_start(h0w[:], h0w_d[:])
            pre = [dma_h2(c) for c in range(H2_CHUNKS)]
            h0 = h0w[:, 0:D]
            w = {name: h0w[:, (1 + i) * D:(2 + i) * D]
                 for i, name in enumerate(W_NAMES)}

            # mean_s1(h1) bank + the final layer-1 bank (self half written
            # mid-stream, neigh half in the tail; the relu reads raw bytes
            # so the two accumulation groups may share the bank).
            ps_mh = psapool.tile([D, BL], F32, tag="ps_mh")
            ps_fin = psapool.tile([D, 2, BL], F32, tag="ps_fin")
            # parity-split relu'd-slice accumulator: (half, parity, batch)
            acc = cpool.tile([D, 2, 2, BL], F16, tag="acc")

            def chunk_mms(c):
                """Self + neigh projections of chunk c into one bank:
                cols = (half, slice, batch)."""
                nsl, s0 = CHUNK_SLS[c], CHUNK_OFF[c]
                ps = pspool.tile([D, 2, 2, BL], F32, tag="ps_sn")
                nc.tensor.matmul(
                    ps[:, 0, 0:nsl, :], lhsT=w8[:, 2, :],
                    rhs=h1t[:, s0:s0 + nsl, :], start=True, stop=True)
                vp = pre[c][:].rearrange("p t s b -> p t (s b)")
                for t in range(5):
                    nc.tensor.matmul(
                        ps[:, 1, 0:nsl, :].rearrange("p s b -> p (s b)"),
                        lhsT=w8[:, 0:2, :], rhs=vp[:, 2 * t:2 * t + 2, :],
                        start=(t == 0), stop=(t == 4), perf_mode=DROW)
                return ps

            def chunk_acc(c, ps):
                """ONE DVE op: acc += relu(psum) (init on chunk 0)."""
                nsl = CHUNK_SLS[c]
                if c == 0:
                    nc.vector.tensor_scalar(
                        acc[:], ps[:], 0.0, None, MAX)
                else:
                    nc.vector.scalar_tensor_tensor(
                        acc[:, :, 0:nsl, :], ps[:, :, 0:nsl, :], 0.0,
                        acc[:, :, 0:nsl, :], MAX, ADD)

            # ---- chunk-paced pipeline ----
            for c in range(H2_CHUNKS):
                ps = chunk_mms(c)
                chunk_acc(c, ps)
                if c == 1:
                    # h1-only work rides between early chunks:
                    # mean_s1(h1) @ wn0 via DoubleRow s1-pairs
                    for t in range(S1 // 2):
                        nc.tensor.matmul(
                            ps_mh[:], lhsT=w8[:, 3:5, :],
                            rhs=h1t[:, 2 * t:2 * t + 2, :],
                            start=(t == 0), stop=False, perf_mode=DROW)
                    nc.tensor.matmul(
                        ps_mh[:], lhsT=w8[:, 3, :], rhs=h1t[:, S1 - 1, :],
                        start=False, stop=True)
                if c == 2:
                    # n0 = relu([h0 @ ws0 ; mean_s1(h1) @ wn0]); ps_0
                    # rotates through a ps_sn slot (no dedicated bank)
                    ps_0 = pspool.tile([D, 2, 2, BL], F32, tag="ps_sn")
                    nc.tensor.matmul(ps_0[:, 0, 0, :], lhsT=w["ws0"],
                                     rhs=h0, start=True, stop=True)
                    n0 = cpool.tile([D, 2, BL], F16, tag="n0")
                    nc.scalar.activation(n0[:, 0, :], ps_0[:, 0, 0, :], RELU)
                    nc.scalar.activation(n0[:, 1, :], ps_mh[:], RELU,
                                         scale=1.0 / SCL)
                if c == 3:
                    # layer-1 self half, mid-stream
                    nc.tensor.matmul(ps_fin[:, 0, :], lhsT=w["ws1a"],
                                     rhs=n0[:, 0, :], start=True, stop=False)
                    nc.tensor.matmul(ps_fin[:, 0, :], lhsT=w["ws1b"],
                                     rhs=n0[:, 1, :], start=False, stop=True)

            # ---- tail: layer-1 neigh half over the 4 accumulator lanes,
            # one relu, one output DMA ----
            nc.tensor.matmul(ps_fin[:, 1, :], lhsT=w["wn1a"],
                             rhs=acc[:, 0, 0, :], start=True, stop=False)
            nc.tensor.matmul(ps_fin[:, 1, :], lhsT=w["wn1a"],
                             rhs=acc[:, 0, 1, :], start=False, stop=False)
            nc.tensor.matmul(ps_fin[:, 1, :], lhsT=w["wn1b"],
                             rhs=acc[:, 1, 0, :], start=False, stop=False)
            nc.tensor.matmul(ps_fin[:, 1, :], lhsT=w["wn1b"],
                             rhs=acc[:, 1, 1, :], start=False, stop=True)
            ofin = cpool.tile([D, 2 * BL], F16, tag="ofin")
            nc.scalar.activation(
                ofin[:], ps_fin[:].rearrange("p a b -> p (a b)"), RELU)
            nc.sync.dma_start(out_d[:], ofin[:])

    nc.compile()
    return nc


def make_in_maps(inputs: dict) -> list[dict]:
    feat = np.asarray(inputs["features"])
    feat16 = feat.astype(np.float16)
    feat8 = feat.astype(ml_dtypes.float8_e4m3)
    s0 = np.asarray(inputs["samples0"]).astype(np.int64).reshape(B)
    s1 = np.asarray(inputs["samples1"]).astype(np.int64).reshape(B, S1)
    s2 = np.asarray(inputs["samples2"]).astype(np.int64).reshape(B, S1, S2)
    ws0 = np.asarray(inputs["w_self0"], dtype=np.float32)
    wn0 = np.asarray(inputs["w_neigh0"], dtype=np.float32)
    ws1 = np.asarray(inputs["w_self1"], dtype=np.float32)
    wn1 = np.asarray(inputs["w_neigh1"], dtype=np.float32)

    # fp16 block order must match W_NAMES; wn1*/(S1*SCL) folds both the
    # s1-mean and the fp8 weight scale (values land in fp16 subnormals,
    # which still carry ~9 bits here).
    w_cat = np.concatenate([
        ws0, ws1[:D], ws1[D:], wn1[:D] / (S1 * SCL), wn1[D:] / (S1 * SCL),
    ], axis=1).astype(np.float16)   # [128, 5*128]
    # fp8 scaled aggregation weights (see build_bass w8 layout)
    wn0s2 = (wn0 * (SCL / S2)).astype(ml_dtypes.float8_e4m3)
    ws0s = (ws0 * SCL).astype(ml_dtypes.float8_e4m3)
    wn0s1 = (wn0 * (SCL / S1)).astype(ml_dtypes.float8_e4m3)
    w8 = np.stack([wn0s2, wn0s2, ws0s, wn0s1, wn0s1], axis=1)  # [128,5,128]
    w8 = np.ascontiguousarray(w8)

    in_maps = []
    for c in range(N_CORES):
        b0 = c * BL
        # h2T: per chunk, cols = (s2, s1_local, batch)
        s2c = s2[b0:b0 + BL]                         # [BL, S1, S2]
        idx_parts = []
        for cc in range(H2_CHUNKS):
            o, nsl = CHUNK_OFF[cc], CHUNK_SLS[cc]
            blk = s2c[:, o:o + nsl, :]               # [BL, nsl, S2]
            idx_parts.append(blk.transpose(2, 1, 0).reshape(-1))
        ids2 = np.concatenate(idx_parts)
        h2T = np.ascontiguousarray(feat8[ids2].T)    # [128, 32000] fp8
        ids1 = s1[b0:b0 + BL].T.reshape(-1)          # (s1, b) flat
        h1T = np.ascontiguousarray(feat8[ids1].T)    # [128, 3200] fp8
        h0T = feat16[s0[b0:b0 + BL]].T               # [128, 128]
        h0w = np.ascontiguousarray(
            np.concatenate([h0T, w_cat], axis=1))    # [128, 768]
        in_maps.append(dict(h2=h2T, h1=h1T, h0w=h0w, w8=w8))
    return in_maps


_NC_CACHE = None


def _get_nc() -> bass.Bass:
    global _NC_CACHE
    if _NC_CACHE is None:
        _NC_CACHE = build_bass()
    return _NC_CACHE


def run(inputs: dict, trace: bool = False):
    """Returns (full_output [1024, 256] f32, BassKernelResults)."""
    in_maps = make_in_maps(inputs)
    res = run_bass_kernel_spmd(
        _get_nc(), in_maps, core_ids=list(range(N_CORES)), trace=trace
    )
    # device out r[j, half*128+b] -> out[b, half*128+j]
    outs = []
    for r in res.results:
        r2 = np.asarray(r["out"], dtype=np.float32).reshape(D, 2, BL)
        outs.append(r2.transpose(2, 1, 0).reshape(BL, 2 * D))
    return np.concatenate(outs, axis=0), res


def kernel(**inputs) -> np.ndarray:
    out, _ = run(inputs)
    return out
